# revision 6
# baseline (speedup 1.0000x reference)
"""Masked self-attention Trainium2 kernel (v3 — streaming, batched normalize).

Reference computes (per batch b):
    key   = x @ Wk.T            [S, 64]
    query = x @ Wq.T            [S, 64]
    value = x @ Wv.T            [S, 128]
    kT_m  = tril(key.T)         [64, S]   -- element (d, s) kept iff s <= d
    out   = softmax(query @ kT_m, axis=-1) @ value

tril zeroes every score column s >= 64, so with a fixed shift c:

    out[i] = (sum_{s<64} e^{z_s - c} v[s]  +  e^{-c} * Vtail) /
             (sum_{s<64} e^{z_s - c}       +  e^{-c} * (S-64))

with Vtail = (sum_{s>=64} x[s]) @ Wv.T.  Per core (batch b, half h):

    zT   = WzaugT.T @ xoT   with Wzaug = [tril_mask(key64) @ Wq | 0-col]
           (zero column -> z row 64 == 0 -> exp emits the e^{-c} row free)
    pT   = exp(zT - c)                      [65, 2048] bf16
    oaug = pT_tile.T @ [v64 | 1 ; vtail | S-64]  (num | den per 128-token tile)
    out  = oaug[:, :128] * (1 / oaug[:, 128])

v3 structure (informed by trace analysis):
- All 8 cores stream HBM concurrently; effective per-core DMA bandwidth is
  ~150 GB/s, so input bytes are minimized: own half fp16 (z needs it), other
  half fp8 (feeds only the Vtail sum, where fp8 noise is ~1e-4 of the
  output).  All transfers span 128 partitions.
- Free-axis column sums only exist on DVE (reduce) and ACT (accum_out), so
  the own-half sums run as four 512-col DVE reduces pipelined behind the
  chunked x DMAs with a running accumulator, and the fp8 half accumulates on
  the scalar engine during an activation copy.  Post-load critical chain is
  one reduce + one add + the vtail matmul + one row copy.
- Out tiles: 4 PSUM groups of 4 tiles (2 banks each, tiles at 0/129/512/641
  so no tile straddles a bank).  Normalize for even groups is two batched
  DVE ops (stride-0 broadcast reciprocal of the 4 den columns, then one
  multiply); odd groups use per-tile DVE reciprocal + scalar-engine scale,
  splitting the load across both PSUM-capable engines.
"""

import numpy as np

import concourse.bass as bass
import concourse.bacc as bacc
import concourse.tile as tile
from concourse import mybir
from concourse.bass_utils import run_bass_kernel_spmd

F32 = mybir.dt.float32
F16 = mybir.dt.float16
BF16 = mybir.dt.bfloat16
FP8 = mybir.dt.float8e4
AF = mybir.ActivationFunctionType
AX = mybir.AxisListType

B, S, E, KD = 4, 4096, 128, 64
HALF = S // 2            # tokens handled per core
NCORES = 8
CHUNK = 512              # tokens per z-matmul / exp / own-sum reduce
NCHUNK = HALF // CHUNK
TSUB = 128               # tokens per output matmul (M <= 128)
NTILE = HALF // TSUB
CSHIFT = 20.0            # fixed softmax shift
NTAIL = float(S - KD)    # 4032 all-zero score columns

X64_OFF, WK_OFF, WV_OFF = 0, KD, 2 * KD
WPE_COLS = 2 * KD + E
WQ_OFF, TRI_OFF = 0, E
WPQ_COLS = E + KD


def _build_nc() -> bass.Bass:
    nc = bacc.Bacc("TRN2", target_bir_lowering=False, debug=False)

    xoT = nc.dram_tensor("xoT", [E, HALF], F16, kind="ExternalInput").ap()
    xo8 = nc.dram_tensor("xo8", [E, HALF], FP8, kind="ExternalInput").ap()
    wpe = nc.dram_tensor("wpe", [E, WPE_COLS], F16, kind="ExternalInput").ap()
    wpq = nc.dram_tensor("wpq", [KD, WPQ_COLS], F16, kind="ExternalInput").ap()
    out = nc.dram_tensor("out", [TSUB, NTILE, E], BF16, kind="ExternalOutput").ap()

    with tile.TileContext(nc) as tc:
        with (
            tc.tile_pool(name="singles", bufs=1) as singles,
            tc.tile_pool(name="pre_ps", bufs=1, space="PSUM") as pre_ps,
            tc.tile_pool(name="z_ps", bufs=2, space="PSUM") as z_ps,
            tc.tile_pool(name="oa_ps", bufs=2, space="PSUM") as oa_ps,
            tc.tile_pool(name="recs", bufs=4) as recs,
            tc.tile_pool(name="obs", bufs=2) as obs,
        ):
            # ---- DMA in.  sync ring: wpe + all own-half x (three chunks so
            # z/reduce work pipelines and the last chunk's post-land chain is
            # short); scalar ring: wpq; gpsimd ring: the fp8 other half.
            wpe_sb = singles.tile([E, WPE_COLS], F16)
            nc.sync.dma_start(wpe_sb[:], wpe)
            wpq_sb = singles.tile([KD, WPQ_COLS], F16)
            nc.scalar.dma_start(wpq_sb[:], wpq)
            xo8_sb = singles.tile([E, HALF], FP8)
            nc.gpsimd.dma_start(xo8_sb[:], xo8)
            xoT_sb = singles.tile([E, HALF], F16)
            nc.sync.dma_start(xoT_sb[:, 0:1024], xoT[:, 0:1024])
            nc.scalar.dma_start(xoT_sb[:, 1024:1536], xoT[:, 1024:1536])
            nc.sync.dma_start(xoT_sb[:, 1536:2048], xoT[:, 1536:2048])

            x64T_sb = wpe_sb[:, X64_OFF : X64_OFF + KD]
            wkT_sb = wpe_sb[:, WK_OFF : WK_OFF + KD]
            wvT_sb = wpe_sb[:, WV_OFF : WV_OFF + E]
            wq_sb = wpq_sb[:, WQ_OFF : WQ_OFF + E]
            tri_sb = wpq_sb[:, TRI_OFF : TRI_OFF + KD]

            # ---- constants (gpsimd is otherwise idle) ----
            wzaug_sb = singles.tile([E, KD + 1], F16)
            nc.gpsimd.memset(wzaug_sb[:, KD : KD + 1], 0.0)
            vaug_sb = singles.tile([KD + 1, E + 1], BF16)
            nc.gpsimd.memset(vaug_sb[0:KD, E : E + 1], 1.0)
            nc.gpsimd.memset(vaug_sb[KD : KD + 1, E : E + 1], NTAIL)
            nbias_sb = singles.tile([KD + 1, 1], F32)
            nc.gpsimd.memset(nbias_sb[:], -CSHIFT)

            # ---- preamble ----
            kT_ps = pre_ps.tile([KD, KD], F32, tag="pre")
            nc.tensor.matmul(kT_ps[:], wkT_sb, x64T_sb, start=True, stop=True)
            kmT_sb = singles.tile([KD, KD], F16)
            nc.vector.tensor_mul(kmT_sb[:], kT_ps[:], tri_sb)

            wzT_ps = pre_ps.tile([E, KD], F32, tag="pre")
            nc.tensor.matmul(wzT_ps[:], wq_sb, kmT_sb[:], start=True, stop=True)
            nc.vector.tensor_copy(wzaug_sb[:, 0:KD], wzT_ps[:])

            v64_ps = pre_ps.tile([KD, E], F32, tag="pre")
            nc.tensor.matmul(v64_ps[:], x64T_sb, wvT_sb, start=True, stop=True)
            nc.vector.tensor_copy(vaug_sb[0:KD, 0:E], v64_ps[:])

            x64s_sb = singles.tile([E, 1], F32)
            nc.vector.reduce_sum(out=x64s_sb[:], in_=x64T_sb, axis=AX.X)

            # ---- z + exp per chunk ----
            pT_sb = singles.tile([KD + 1, HALF], BF16)
            for c in range(NCHUNK):
                cs = slice(c * CHUNK, (c + 1) * CHUNK)
                zaug_ps = z_ps.tile([KD + 1, CHUNK], F32, tag="z")
                nc.tensor.matmul(
                    zaug_ps[:], wzaug_sb[:], xoT_sb[:, cs], start=True, stop=True
                )
                nc.scalar.activation(
                    pT_sb[0 : KD + 1, cs], zaug_ps[:], AF.Exp, bias=nbias_sb[:]
                )

            # ---- batch tail column-sum -> vtail row of vaug ----
            # fp8 other half accumulates on the scalar engine; own half as
            # four DVE reduces with a running sum so only the last 512
            # columns sit on the post-land critical chain.
            r8_sb = singles.tile([E, 1], F32)
            scr_sb = singles.tile([E, HALF], F16)
            nc.scalar.activation(
                scr_sb[:], xo8_sb[:], AF.Copy, accum_out=r8_sb[:]
            )
            rd_sb = [
                singles.tile([E, 1], F32, name=f"rd{c}_sb") for c in range(NCHUNK)
            ]
            for c in range(NCHUNK):
                cs = slice(c * CHUNK, (c + 1) * CHUNK)
                nc.vector.reduce_sum(out=rd_sb[c][:], in_=xoT_sb[:, cs], axis=AX.X)
            acc01_sb = singles.tile([E, 1], F32)
            nc.vector.tensor_add(acc01_sb[:], rd_sb[0][:], rd_sb[1][:])
            acc012_sb = singles.tile([E, 1], F32)
            nc.vector.tensor_add(acc012_sb[:], acc01_sb[:], rd_sb[2][:])
            # base = r8 + acc012 - x64s  (everything except the last chunk)
            b0_sb = singles.tile([E, 1], F32)
            nc.vector.tensor_sub(b0_sb[:], r8_sb[:], x64s_sb[:])
            base_sb = singles.tile([E, 1], F32)
            nc.vector.tensor_add(base_sb[:], b0_sb[:], acc012_sb[:])
            tailh_sb = singles.tile([E, 1], F16)
            nc.vector.tensor_add(tailh_sb[:], base_sb[:], rd_sb[3][:])
            vtail_ps = pre_ps.tile([1, E], F32, tag="pre")
            nc.tensor.matmul(vtail_ps[:], tailh_sb[:], wvT_sb, start=True, stop=True)
            nc.vector.tensor_copy(vaug_sb[KD : KD + 1, 0:E], vtail_ps[:])

            # ---- out tiles: 4 PSUM groups of 4 ----
            out_engs = (nc.sync, nc.gpsimd, nc.sync, nc.gpsimd)
            for g in range(4):
                # 2 banks; tiles j at col offsets {0,129} in bank 0 and
                # {0,129} in bank 1 -> view [128][bank=2][tile=2][129]
                G = oa_ps.tile([TSUB, 2, 512], F32, tag="oa")
                G4 = G[:, :, 0:258].rearrange("p b (t x) -> p b t x", t=2)
                for j in range(4):
                    t = 4 * g + j
                    ts = slice(t * TSUB, (t + 1) * TSUB)
                    nc.tensor.matmul(
                        G4[:, j // 2, j % 2, :],
                        pT_sb[0 : KD + 1, ts],
                        vaug_sb[:],
                        start=True,
                        stop=True,
                    )
                ob_sb = obs.tile([TSUB, 4, E], BF16, tag="ob")
                ob4 = ob_sb[:].rearrange("p (b t) x -> p b t x", b=2)
                nums = G4[:, :, :, 0:E]
                dens = G4[:, :, :, E : E + 1]
                if g % 2 == 0:
                    # batched: one broadcast reciprocal + one multiply
                    rb_sb = recs.tile([TSUB, 4, E], F32, tag="rb")
                    rb4 = rb_sb[:].rearrange("p (b t) x -> p b t x", b=2)
                    nc.vector.reciprocal(rb4, dens.broadcast_to([TSUB, 2, 2, E]))
                    nc.vector.tensor_mul(ob4, nums, rb4)
                else:
                    # per-tile: DVE reciprocal + scalar-engine scale
                    for j in range(4):
                        rec_sb = recs.tile([TSUB, 1], F32, tag="rec")
                        nc.vector.reciprocal(rec_sb[:], dens[:, j // 2, j % 2, :])
                        nc.scalar.activation(
                            ob4[:, j // 2, j % 2, :],
                            nums[:, j // 2, j % 2, :],
                            AF.Copy,
                            scale=rec_sb[:],
                        )
                out_engs[g].dma_start(out[:, 4 * g : 4 * g + 4, :], ob_sb[:])

    nc.compile()
    return nc


_NC_CACHE = None


def _get_nc() -> bass.Bass:
    global _NC_CACHE
    if _NC_CACHE is None:
        _NC_CACHE = _build_nc()
    return _NC_CACHE


def _make_in_maps(x, Wk, Wq, Wv):
    tri = (np.arange(KD)[:, None] >= np.arange(KD)[None, :]).astype(np.float16)
    wpq = np.concatenate([Wq.astype(np.float16), tri], axis=1)
    wpq = np.ascontiguousarray(wpq)
    x16 = x.astype(np.float16)
    fp8_np = mybir.dt.np(FP8)
    in_maps = []
    for c in range(NCORES):
        b, h = divmod(c, 2)
        xb = x16[b]
        wpe = np.concatenate(
            [xb[:KD].T, Wk.T.astype(np.float16), Wv.T.astype(np.float16)], axis=1
        )
        own = xb[h * HALF : (h + 1) * HALF]
        other = xb[(1 - h) * HALF : (2 - h) * HALF]
        in_maps.append(
            {
                "xoT": np.ascontiguousarray(own.T),
                "xo8": np.ascontiguousarray(other.T.astype(fp8_np)),
                "wpe": np.ascontiguousarray(wpe),
                "wpq": wpq,
            }
        )
    return in_maps


def _gather(results):
    out = np.empty((B, S, E), np.float32)
    for c, r in enumerate(results):
        b, h = divmod(c, 2)
        # device layout [p, t, v], token = t*128 + p
        dev = np.asarray(r["out"], dtype=np.float32)
        out[b, h * HALF : (h + 1) * HALF] = dev.transpose(1, 0, 2).reshape(HALF, E)
    return out


def _run(x, Wk, Wq, Wv, **spmd_kwargs):
    nc = _get_nc()
    res = run_bass_kernel_spmd(
        nc,
        _make_in_maps(x, Wk, Wq, Wv),
        core_ids=list(range(NCORES)),
        **spmd_kwargs,
    )
    return _gather(res.results), res


def kernel(x, Wk, Wq, Wv):
    x = np.ascontiguousarray(np.asarray(x), dtype=np.float32)
    Wk = np.ascontiguousarray(np.asarray(Wk), dtype=np.float32)
    Wq = np.ascontiguousarray(np.asarray(Wq), dtype=np.float32)
    Wv = np.ascontiguousarray(np.asarray(Wv), dtype=np.float32)
    out, _ = _run(x, Wk, Wq, Wv)
    return out


# revision 8
# speedup vs baseline: 1.3127x; 1.3127x over previous
"""Masked self-attention Trainium2 kernel (v4).

Reference computes (per batch b):
    key   = x @ Wk.T            [S, 64]
    query = x @ Wq.T            [S, 64]
    value = x @ Wv.T            [S, 128]
    kT_m  = tril(key.T)         [64, S]   -- element (d, s) kept iff s <= d
    out   = softmax(query @ kT_m, axis=-1) @ value

tril zeroes every score column s >= 64, so with a fixed shift c:

    out[i] = (sum_{s<64} e^{z_s - c} v[s]  +  e^{-c} * Vtail) /
             (sum_{s<64} e^{z_s - c}       +  e^{-c} * (S-64))

with Vtail = (sum_{s>=64} x[s]) @ Wv.T.  Per core (batch b, half h):

    zT   = WzaugT.T @ xoT   with Wzaug = [tril_mask(key64) @ Wq | 0-col]
           (zero column -> z row 64 == 0 -> exp emits the e^{-c} row free)
    pT   = exp(zT - c)                      [65, 2048] bf16
    oaug = pT_tile.T @ [v64 | 1 ; vtail | S-64]  (num | den per 128-token tile)
    out  = oaug[:, :128] * (1 / oaug[:, 128])

Hard-won trace lessons baked into v4:
- Every dma_start moves one fully-CONTIGUOUS DRAM region (column-sliced
  views of a wide tensor read HBM at 2KB-strided-by-4KB and halve effective
  bandwidth), so the host packs one DRAM tensor per transfer: merged
  weights, three own-half x chunks, the fp8 other half, and four output
  chunks.
- All 8 cores stream concurrently; per-core HBM bandwidth is ~150 GB/s, so
  input bytes are minimized: own half fp16 (z needs it), other half fp8
  (feeds only the Vtail sum, where fp8 noise is ~1e-4 of the output).
- Free-axis column sums only exist on DVE (reduce) and ACT (accum_out): own
  half as four 512-col DVE reduces with a running accumulator (only the
  last chunk's reduce sits on the post-load chain), fp8 half via scalar
  activation accum during the load.
- Stride-0 broadcast APs on DVE are ~9x slow — normalize is per-tile:
  DVE reciprocal + scale alternating between DVE and the scalar engine.
"""

import numpy as np

import concourse.bass as bass
import concourse.bacc as bacc
import concourse.tile as tile
from concourse import mybir
from concourse.bass_utils import run_bass_kernel_spmd

F32 = mybir.dt.float32
F16 = mybir.dt.float16
BF16 = mybir.dt.bfloat16
FP8 = mybir.dt.float8e4
AF = mybir.ActivationFunctionType
AX = mybir.AxisListType

B, S, E, KD = 4, 4096, 128, 64
HALF = S // 2            # tokens handled per core
NCORES = 8
CHUNK = 512              # tokens per z-matmul / exp / own-sum reduce
NCHUNK = HALF // CHUNK
TSUB = 128               # tokens per output matmul (M <= 128)
NTILE = HALF // TSUB
CSHIFT = 20.0            # fixed softmax shift
NTAIL = float(S - KD)    # 4032 all-zero score columns

# Merged weight pack [128, 448]: [x64T | WkT | WvT | Wq-pad | tri-pad]
X64_OFF, WK_OFF, WV_OFF, WQ_OFF, TRI_OFF = 0, KD, 2 * KD, 2 * KD + E, 2 * KD + 2 * E
WPK_COLS = 3 * KD + 2 * E  # 448


def _build_nc() -> bass.Bass:
    nc = bacc.Bacc("TRN2", target_bir_lowering=False, debug=False)

    wpk = nc.dram_tensor("wpk", [E, WPK_COLS], F16, kind="ExternalInput").ap()
    xa = nc.dram_tensor("xa", [E, 1024], F16, kind="ExternalInput").ap()
    xb = nc.dram_tensor("xb", [E, 512], F16, kind="ExternalInput").ap()
    xc = nc.dram_tensor("xc", [E, 512], F16, kind="ExternalInput").ap()
    xo8 = nc.dram_tensor("xo8", [E, HALF], FP8, kind="ExternalInput").ap()
    outs = [
        nc.dram_tensor(f"o{g}", [TSUB, 4, E], BF16, kind="ExternalOutput").ap()
        for g in range(4)
    ]

    with tile.TileContext(nc) as tc:
        with (
            tc.tile_pool(name="singles", bufs=1) as singles,
            tc.tile_pool(name="pre_ps", bufs=1, space="PSUM") as pre_ps,
            tc.tile_pool(name="z_ps", bufs=2, space="PSUM") as z_ps,
            tc.tile_pool(name="oa_ps", bufs=4, space="PSUM") as oa_ps,
            tc.tile_pool(name="recs", bufs=4) as recs,
            tc.tile_pool(name="obs", bufs=2) as obs,
        ):
            # ---- DMA in: each transfer is one contiguous DRAM tensor ----
            wpk_sb = singles.tile([E, WPK_COLS], F16)
            nc.sync.dma_start(wpk_sb[:], wpk)
            xo8_sb = singles.tile([E, HALF], FP8)
            nc.gpsimd.dma_start(xo8_sb[:], xo8)
            xoT_sb = singles.tile([E, HALF], F16)
            nc.sync.dma_start(xoT_sb[:, 0:1024], xa)
            nc.scalar.dma_start(xoT_sb[:, 1024:1536], xb)
            nc.scalar.dma_start(xoT_sb[:, 1536:2048], xc)

            x64T_sb = wpk_sb[:, X64_OFF : X64_OFF + KD]
            wkT_sb = wpk_sb[:, WK_OFF : WK_OFF + KD]
            wvT_sb = wpk_sb[:, WV_OFF : WV_OFF + E]
            wq_sb = wpk_sb[0:KD, WQ_OFF : WQ_OFF + E]
            tri_sb = wpk_sb[0:KD, TRI_OFF : TRI_OFF + KD]

            # ---- constants (gpsimd is otherwise idle) ----
            wzaug_sb = singles.tile([E, KD + 1], F16)
            nc.gpsimd.memset(wzaug_sb[:, KD : KD + 1], 0.0)
            vaug_sb = singles.tile([KD + 1, E + 1], BF16)
            nc.gpsimd.memset(vaug_sb[0:KD, E : E + 1], 1.0)
            nc.gpsimd.memset(vaug_sb[KD : KD + 1, E : E + 1], NTAIL)
            nbias_sb = singles.tile([KD + 1, 1], F32)
            nc.gpsimd.memset(nbias_sb[:], -CSHIFT)

            # ---- preamble ----
            kT_ps = pre_ps.tile([KD, KD], F32, tag="pre")
            nc.tensor.matmul(kT_ps[:], wkT_sb, x64T_sb, start=True, stop=True)
            kmT_sb = singles.tile([KD, KD], F16)
            nc.vector.tensor_mul(kmT_sb[:], kT_ps[:], tri_sb)

            wzT_ps = pre_ps.tile([E, KD], F32, tag="pre")
            nc.tensor.matmul(wzT_ps[:], wq_sb, kmT_sb[:], start=True, stop=True)
            nc.vector.tensor_copy(wzaug_sb[:, 0:KD], wzT_ps[:])

            v64_ps = pre_ps.tile([KD, E], F32, tag="pre")
            nc.tensor.matmul(v64_ps[:], x64T_sb, wvT_sb, start=True, stop=True)
            nc.vector.tensor_copy(vaug_sb[0:KD, 0:E], v64_ps[:])

            x64s_sb = singles.tile([E, 1], F32)
            nc.vector.reduce_sum(out=x64s_sb[:], in_=x64T_sb, axis=AX.X)

            # ---- z + exp per chunk ----
            pT_sb = singles.tile([KD + 1, HALF], BF16)
            for c in range(NCHUNK):
                cs = slice(c * CHUNK, (c + 1) * CHUNK)
                zaug_ps = z_ps.tile([KD + 1, CHUNK], F32, tag="z")
                nc.tensor.matmul(
                    zaug_ps[:], wzaug_sb[:], xoT_sb[:, cs], start=True, stop=True
                )
                nc.scalar.activation(
                    pT_sb[0 : KD + 1, cs], zaug_ps[:], AF.Exp, bias=nbias_sb[:]
                )

            # ---- batch tail column-sum -> vtail row of vaug ----
            r8_sb = singles.tile([E, 1], F32)
            scr_sb = singles.tile([E, HALF], F16)
            nc.scalar.activation(scr_sb[:], xo8_sb[:], AF.Copy, accum_out=r8_sb[:])
            rd_sb = [
                singles.tile([E, 1], F32, name=f"rd{c}_sb") for c in range(NCHUNK)
            ]
            for c in range(NCHUNK):
                cs = slice(c * CHUNK, (c + 1) * CHUNK)
                nc.vector.reduce_sum(out=rd_sb[c][:], in_=xoT_sb[:, cs], axis=AX.X)
            acc01_sb = singles.tile([E, 1], F32)
            nc.vector.tensor_add(acc01_sb[:], rd_sb[0][:], rd_sb[1][:])
            acc012_sb = singles.tile([E, 1], F32)
            nc.vector.tensor_add(acc012_sb[:], acc01_sb[:], rd_sb[2][:])
            b0_sb = singles.tile([E, 1], F32)
            nc.vector.tensor_sub(b0_sb[:], r8_sb[:], x64s_sb[:])
            base_sb = singles.tile([E, 1], F32)
            nc.vector.tensor_add(base_sb[:], b0_sb[:], acc012_sb[:])
            tailh_sb = singles.tile([E, 1], F16)
            nc.vector.tensor_add(tailh_sb[:], base_sb[:], rd_sb[3][:])
            vtail_ps = pre_ps.tile([1, E], F32, tag="pre")
            nc.tensor.matmul(vtail_ps[:], tailh_sb[:], wvT_sb, start=True, stop=True)
            nc.vector.tensor_copy(vaug_sb[KD : KD + 1, 0:E], vtail_ps[:])

            # ---- out tiles ----
            out_engs = (nc.sync, nc.gpsimd, nc.sync, nc.gpsimd)
            for t in range(NTILE):
                if t % 4 == 0:
                    ob_sb = obs.tile([TSUB, 4, E], BF16, tag="ob")
                ts = slice(t * TSUB, (t + 1) * TSUB)
                oa = oa_ps.tile([TSUB, E + 1], F32, tag="oa")
                nc.tensor.matmul(
                    oa[:], pT_sb[0 : KD + 1, ts], vaug_sb[:], start=True, stop=True
                )
                rec_sb = recs.tile([TSUB, 1], F32, tag="rec")
                nc.vector.reciprocal(rec_sb[:], oa[:, E : E + 1])
                if t % 2 == 0:
                    nc.scalar.activation(
                        ob_sb[:, t % 4, :], oa[:, 0:E], AF.Copy, scale=rec_sb[:]
                    )
                else:
                    nc.vector.tensor_scalar_mul(
                        ob_sb[:, t % 4, :], oa[:, 0:E], rec_sb[:]
                    )
                if t % 4 == 3:
                    g = t // 4
                    out_engs[g].dma_start(outs[g], ob_sb[:])

    nc.compile()
    return nc


_NC_CACHE = None


def _get_nc() -> bass.Bass:
    global _NC_CACHE
    if _NC_CACHE is None:
        _NC_CACHE = _build_nc()
    return _NC_CACHE


def _make_in_maps(x, Wk, Wq, Wv):
    tri = (np.arange(KD)[:, None] >= np.arange(KD)[None, :]).astype(np.float16)
    wq_pad = np.zeros((E, E), np.float16)
    wq_pad[:KD] = Wq.astype(np.float16)
    tri_pad = np.zeros((E, KD), np.float16)
    tri_pad[:KD] = tri
    x16 = x.astype(np.float16)
    fp8_np = mybir.dt.np(FP8)
    in_maps = []
    for c in range(NCORES):
        b, h = divmod(c, 2)
        xb_ = x16[b]
        wpk = np.concatenate(
            [
                xb_[:KD].T,
                Wk.T.astype(np.float16),
                Wv.T.astype(np.float16),
                wq_pad,
                tri_pad,
            ],
            axis=1,
        )
        own = xb_[h * HALF : (h + 1) * HALF].T  # [E, 2048]
        other = xb_[(1 - h) * HALF : (2 - h) * HALF].T
        in_maps.append(
            {
                "wpk": np.ascontiguousarray(wpk),
                "xa": np.ascontiguousarray(own[:, 0:1024]),
                "xb": np.ascontiguousarray(own[:, 1024:1536]),
                "xc": np.ascontiguousarray(own[:, 1536:2048]),
                "xo8": np.ascontiguousarray(other.astype(fp8_np)),
            }
        )
    return in_maps


def _gather(results):
    out = np.empty((B, S, E), np.float32)
    for c, r in enumerate(results):
        b, h = divmod(c, 2)
        # per-group device layout [p, t, v], token = (4g + t)*128 + p
        dev = np.concatenate(
            [np.asarray(r[f"o{g}"], dtype=np.float32) for g in range(4)], axis=1
        )
        out[b, h * HALF : (h + 1) * HALF] = dev.transpose(1, 0, 2).reshape(HALF, E)
    return out


def _run(x, Wk, Wq, Wv, **spmd_kwargs):
    nc = _get_nc()
    res = run_bass_kernel_spmd(
        nc,
        _make_in_maps(x, Wk, Wq, Wv),
        core_ids=list(range(NCORES)),
        **spmd_kwargs,
    )
    return _gather(res.results), res


def kernel(x, Wk, Wq, Wv):
    x = np.ascontiguousarray(np.asarray(x), dtype=np.float32)
    Wk = np.ascontiguousarray(np.asarray(Wk), dtype=np.float32)
    Wq = np.ascontiguousarray(np.asarray(Wq), dtype=np.float32)
    Wv = np.ascontiguousarray(np.asarray(Wv), dtype=np.float32)
    out, _ = _run(x, Wk, Wq, Wv)
    return out
